# revision 1
# baseline (speedup 1.0000x reference)
"""Box filter (radius 8, window 17, zero-padded edges) over dims 2,3 of a
[8, 32, 512, 512] f32 tensor, on 8 Trainium2 NeuronCores.

Decomposition (validated vs the jax reference):
  - Per-axis filter with clipped windows = multiplication by a banded ones
    matrix B (B[i,k] = 1 iff |i-k| <= 8), i.e. Z = B @ X @ B.
  - Column (free-dim) filter: one fused DVE `tensor_tensor_scan` per
    128-row block computes the sliding-window sum via the recurrence
        state[t] = (x[t] + state[t-1]) - x[t-17]
    over a zero-padded buffer (17 zeros in front, 8 behind); scan output
    position c+8 holds the window centered at c. The scan downcasts its
    fp32 state to bf16 on store, so the row matmul runs 1-pass.
  - Row (partition-dim) filter: PE matmul time is ~605 ns per 512-column
    pump regardless of K, so the layout minimizes matmul COUNT: block t
    holds input rows 128t-8..128t+119 (8-row top halo pre-shifted into the
    block), so one K=128 banded matmul covers an output tile except its
    last 16 rows, which one K=16 bottom-fix matmul (rows 120..135 from the
    next block) accumulates into the same PSUM bank. 8 matmuls/channel.
  - Rows 504..511 don't fit in the shifted blocks; they are stashed in
    block 0's partitions 0..7 (which would otherwise hold nonexistent rows
    -8..-1), scanned along with block 0, and consumed by tile 3's fix.
    Tile 0's main uses a lhsT with rows 0..7 zeroed to ignore the stash.

Every input row is loaded and scanned exactly once: HBM traffic is the
roofline-minimal 32 MB in + 32 MB out per core, the DVE does 4 scans per
channel, the PE 8 matmuls.

Sharding: data-parallel over batch (dim 0) -> 8 cores, one batch each.
"""

import os
import sys

import numpy as np

for _p in ("/opt/trn_rl_repo", "/root/.axon_site/_ro/trn_rl_repo"):
    if os.path.isdir(_p) and _p not in sys.path:
        sys.path.append(_p)

import ml_dtypes

import concourse.bass as bass
import concourse.tile as tile
from concourse import bacc, mybir
from concourse.bass_utils import run_bass_kernel_spmd

R = 8
PADF = 2 * R + 1  # front zero pad (window width)
PADB = R          # back zero pad
H = W = 512
CH = 32
NCORES = 8
XW = PADF + W + PADB  # 537
XALL = 4 * XW         # 2148
NBIG = 4
NOBIG = 3

_CACHE = {}


def _banded():
    # Block t partition k holds input row 128t - 8 + k. Main band:
    # |m - (k - 8)| <= 8  ->  k - 16 <= m <= k.
    k = np.arange(128)[:, None]
    m = np.arange(128)[None, :]
    bmain = ((m >= k - 2 * R) & (m <= k)).astype(np.float32)
    # Tile 0's main: partitions 0..7 hold the stashed rows 504..511, not
    # rows -8..-1 — zero them out (zero-pad semantics at the top edge).
    bmainf = bmain.copy()
    bmainf[0:R, :] = 0.0
    # Bottom fix for tile t<3: rhs = ub[t+1] partitions k (k=0..15) =
    # input row 128t + 120 + k; output rows 64+m (m=0..63, psum slice
    # [64:128]): window iff m + 64 >= k + 112  ->  m >= k + 48.
    kb = np.arange(16)[:, None]
    mb = np.arange(64)[None, :]
    bbot = (mb >= kb + 6 * R).astype(np.float32)
    # Tile 3's fix: rhs = ub[0] partitions k (k=0..7) = stashed row 504+k;
    # same band m >= k + 48.
    kl = np.arange(8)[:, None]
    bbotl = (mb >= kl + 6 * R).astype(np.float32)
    bf = ml_dtypes.bfloat16
    return (bmainf.astype(bf), bmain.astype(bf),
            bbot.astype(bf), bbotl.astype(bf))


def _build_program():
    if "nc" in _CACHE:
        return _CACHE["nc"]
    # Bacc (not raw Bass): its compile() legalizes sync waits — TRN2 allows
    # at most 1 wait per instruction; excess waits become standalone
    # EventSemaphore instructions (and matmul waits move to ldweights).
    nc = bacc.Bacc(debug=False)
    f32 = mybir.dt.float32
    bf16 = mybir.dt.bfloat16
    x = nc.dram_tensor("x", [CH, H, W], f32, kind="ExternalInput")
    z = nc.dram_tensor("z", [CH, H, W], f32, kind="ExternalOutput")
    bma = nc.dram_tensor("bma", [128, 128], bf16, kind="ExternalInput")
    bmb = nc.dram_tensor("bmb", [128, 128], bf16, kind="ExternalInput")
    bb2 = nc.dram_tensor("bb2", [16, 64], bf16, kind="ExternalInput")
    bbl = nc.dram_tensor("bbl", [8, 64], bf16, kind="ExternalInput")
    xap, zap = x.ap(), z.ap()

    with tile.TileContext(nc) as tc:
        with (
            tc.tile_pool(name="consts", bufs=1) as cpool,
            tc.tile_pool(name="ubuf", bufs=8) as upool,
            tc.tile_pool(name="psum", bufs=8, space="PSUM") as ppool,
        ):
            bmat = cpool.tile([128, 128], bf16)
            bmbt = cpool.tile([128, 128], bf16)
            bb2t = cpool.tile([16, 64], bf16)
            bblt = cpool.tile([8, 64], bf16)
            # consts on the (otherwise idle at the head) scalar queue so
            # channel 0's loads are the first sync-queue triggers
            nc.scalar.dma_start(bmat[:], bma.ap()[:, :])
            nc.scalar.dma_start(bmbt[:], bmb.ap()[:, :])
            nc.scalar.dma_start(bb2t[:], bb2.ap()[:, :])
            nc.scalar.dma_start(bblt[:], bbl.ap()[:, :])

            # Static ring. Loads only ever touch the data columns, so the
            # 17+8 zero pads around each 512-col block are zeroed ONCE here
            # — three tiny strided memsets per buffer (~100 ns each)
            # instead of a full-buffer clear.
            xalls = [
                nc.alloc_sbuf_tensor(f"xall{i}", [128, XALL], f32).ap()
                for i in range(NBIG)
            ]
            for xa in xalls:
                nc.vector.memset(xa[:, 0:PADF], 0.0)
                mid = bass.AP(
                    tensor=xa.tensor,
                    offset=xa.offset + XW - PADB,
                    ap=[[XALL, 128], [XW, 3], [1, PADF + PADB]],
                )
                nc.vector.memset(mid, 0.0)
                nc.vector.memset(xa[:, XALL - PADB:XALL], 0.0)
            obigs = [
                nc.alloc_sbuf_tensor(f"obig{i}", [128, 4, W], f32).ap()
                for i in range(NOBIG)
            ]

            for c in range(CH):
                xa = xalls[c % NBIG]
                og = obigs[c % NOBIG]

                # Three loads per channel (shifted-block layout):
                #   rows 0..119   -> block 0 partitions 8..127
                #   rows 504..511 -> block 0 partitions 0..7 (stash)
                #   rows 120..503 -> blocks 1..3 (partition k = row
                #                    128b - 8 + k), one batched transfer
                nc.sync.dma_start(
                    xa[8:128, PADF:PADF + W], xap[c, 0:120, :]
                )
                nc.sync.dma_start(
                    xa[0:8, PADF:PADF + W], xap[c, 504:512, :]
                )
                src = bass.AP(
                    tensor=x,
                    offset=(c * H + 120) * W,
                    ap=[[W, 128], [128 * W, 3], [1, W]],
                )
                dst = bass.AP(
                    tensor=xa.tensor,
                    offset=xa.offset + XW + PADF,
                    ap=[[XALL, 128], [XW, 3], [1, W]],
                )
                nc.sync.dma_start(dst, src)

                ub = [None] * 4

                def scan(b, xa=xa, ub=ub):
                    u = upool.tile([128, W + PADB], bf16)
                    nc.vector.tensor_tensor_scan(
                        out=u[0:128, :],
                        data0=xa[0:128, b * XW + PADF:b * XW + XW],
                        data1=xa[0:128, b * XW:b * XW + W + PADB],
                        initial=0.0,
                        op0=mybir.AluOpType.add,
                        op1=mybir.AluOpType.subtract,
                    )
                    ub[b] = u

                def tilegroup(t, c=c, og=og, ub=ub):
                    ps = ppool.tile([128, W], f32)
                    nc.tensor.matmul(
                        ps[0:128, :],
                        (bmat if t == 0 else bmbt)[0:128, 0:128],
                        ub[t][0:128, R:R + W],
                        start=True, stop=False, skip_group_check=True,
                    )
                    if t < 3:
                        nc.tensor.matmul(
                            ps[64:128, :], bb2t[0:16, 0:64],
                            ub[t + 1][0:16, R:R + W],
                            start=False, stop=True, skip_group_check=True,
                        )
                    else:
                        nc.tensor.matmul(
                            ps[64:128, :], bblt[0:8, 0:64],
                            ub[0][0:8, R:R + W],
                            start=False, stop=True, skip_group_check=True,
                        )
                    nc.scalar.copy(og[:, t, :], ps[0:128, :])
                    if c == CH - 1:
                        # last channel: per-tile stores so the kernel tail
                        # ends on a small transfer
                        nc.scalar.dma_start(
                            zap[c, 128 * t:128 * t + 128, :], og[:, t, :]
                        )

                scan(0)
                scan(1)
                tilegroup(0)
                scan(2)
                tilegroup(1)
                scan(3)
                tilegroup(2)
                tilegroup(3)

                if c < CH - 1:
                    # ONE batched 1 MB store; follows the copies on the
                    # scalar queue in program order (no extra waits)
                    nc.scalar.dma_start(
                        zap[c, :, :].rearrange("(t p) w -> p t w", p=128),
                        og[:, :, :],
                    )

    nc.compile()
    _CACHE["nc"] = nc
    return nc


def kernel(tensor: np.ndarray) -> np.ndarray:
    tensor = np.ascontiguousarray(np.asarray(tensor, dtype=np.float32))
    assert tensor.shape == (NCORES, CH, H, W)
    bmaf, bmab, bb2, bbl = _banded()
    nc = _build_program()
    in_maps = [
        {"x": tensor[i], "bma": bmaf, "bmb": bmab, "bb2": bb2, "bbl": bbl}
        for i in range(NCORES)
    ]
    res = run_bass_kernel_spmd(nc, in_maps, core_ids=list(range(NCORES)))
    return np.stack([res.results[i]["z"] for i in range(NCORES)], axis=0)



# revision 3
# speedup vs baseline: 1.1325x; 1.1325x over previous
"""Box filter (radius 8, window 17, zero-padded edges) over dims 2,3 of a
[8, 32, 512, 512] f32 tensor, on 8 Trainium2 NeuronCores.

Decomposition (validated vs the jax reference):
  - Per-axis filter with clipped windows = multiplication by a banded ones
    matrix B (B[i,k] = 1 iff |i-k| <= 8), i.e. Z = B @ X @ B.
  - Column (free-dim) filter: DVE `tensor_tensor_scan` computes the sliding
    window sum via state[t] = (x[t] + state[t-1]) - x[t-17] over a buffer
    with 17 front zeros / 8 back zeros per 512-col block. Measured HW cost
    is 55 + 2.09*N ns (dtype-independent), so the 4 blocks of a channel are
    chained into TWO wide scans (blocks 0-1, 2-3): the >=17 zeros between
    blocks drain the window state exactly, so one scan instruction yields
    all blocks' outputs at minimal per-instruction overhead.
  - Row (partition-dim) filter: PE matmul. Block t holds input rows
    128t-8..128t+119 (top halo pre-shifted in), so one K=128 banded matmul
    covers an output tile except its last 16 rows; one K<=16 fix matmul
    accumulates those from the next block's partitions 0..15. Rows 504..511
    are stashed in block 0's partitions 0..7 and consumed by tile 3's fix
    (tile 0's main lhsT has rows 0..7 zeroed to ignore the stash).
  - Everything is bf16 end-to-end (input converted on host, output converted
    back): HBM traffic is halved to 16.8 MB in + 16.8 MB out per core, scan
    state stays fp32 inside the DVE, PSUM accumulates f32, and the Scalar
    engine downcasts PSUM->SBUF. Measured rel err ~3e-3 vs the 2e-2 gate.

Engine budget per channel (measured unit costs): DVE 2 scans ~4.5 us
(critical path), PE 8 matmuls 1.7-5 us (p-state dependent), Scalar 4 copies
~2.8 us, DMA 2.9 us across 16 queues. Stores issue on the sync queue (not
Scalar) to keep the Scalar sequencer clear; pad memsets run on GpSimd.

Sharding: data-parallel over batch (dim 0) -> 8 cores, one batch each.
"""

import os
import sys

import numpy as np

for _p in ("/opt/trn_rl_repo", "/root/.axon_site/_ro/trn_rl_repo"):
    if os.path.isdir(_p) and _p not in sys.path:
        sys.path.append(_p)

import ml_dtypes

import concourse.bass as bass
import concourse.tile as tile
from concourse import bacc, mybir
from concourse.bass_utils import run_bass_kernel_spmd

R = 8
PADF = 2 * R + 1  # front zero pad (window width)
PADB = R          # back zero pad
H = W = 512
CH = 32
NCORES = 8
XW = PADF + W + PADB  # 537
XALL = 4 * XW         # 2148
UW = 4 * XW - PADF    # 2131: merged scan output width
NBIG = 4
NU = 3
NOBIG = 3

_CACHE = {}


def _banded():
    # Block t partition k holds input row 128t - 8 + k. Main band:
    # |m - (k - 8)| <= 8  ->  k - 16 <= m <= k.
    k = np.arange(128)[:, None]
    m = np.arange(128)[None, :]
    bmain = ((m >= k - 2 * R) & (m <= k)).astype(np.float32)
    # Tile 0's main: partitions 0..7 hold the stashed rows 504..511, not
    # rows -8..-1 — zero them out (zero-pad semantics at the top edge).
    bmainf = bmain.copy()
    bmainf[0:R, :] = 0.0
    # Bottom fix for tile t<3: rhs partitions k (k=0..15) = input row
    # 128t + 120 + k; output rows 64+m (m=0..63, psum slice [64:128]):
    # window iff m + 64 >= k + 112  ->  m >= k + 48.
    kb = np.arange(16)[:, None]
    mb = np.arange(64)[None, :]
    bbot = (mb >= kb + 6 * R).astype(np.float32)
    # Tile 3's fix: rhs partitions k (k=0..7) = stashed row 504+k; same band.
    kl = np.arange(8)[:, None]
    bbotl = (mb >= kl + 6 * R).astype(np.float32)
    bf = ml_dtypes.bfloat16
    return (bmainf.astype(bf), bmain.astype(bf),
            bbot.astype(bf), bbotl.astype(bf))


def _build_program():
    if "nc" in _CACHE:
        return _CACHE["nc"]
    nc = bacc.Bacc(debug=False)
    f32 = mybir.dt.float32
    bf16 = mybir.dt.bfloat16
    x = nc.dram_tensor("x", [CH, H, W], bf16, kind="ExternalInput")
    z = nc.dram_tensor("z", [CH, H, W], bf16, kind="ExternalOutput")
    bma = nc.dram_tensor("bma", [128, 128], bf16, kind="ExternalInput")
    bmb = nc.dram_tensor("bmb", [128, 128], bf16, kind="ExternalInput")
    bb2 = nc.dram_tensor("bb2", [16, 64], bf16, kind="ExternalInput")
    bbl = nc.dram_tensor("bbl", [8, 64], bf16, kind="ExternalInput")
    xap, zap = x.ap(), z.ap()

    with tile.TileContext(nc) as tc:
        with (
            tc.tile_pool(name="consts", bufs=1) as cpool,
            tc.tile_pool(name="psum", bufs=8, space="PSUM") as ppool,
        ):
            bmat = cpool.tile([128, 128], bf16)
            bmbt = cpool.tile([128, 128], bf16)
            bb2t = cpool.tile([16, 64], bf16)
            bblt = cpool.tile([8, 64], bf16)
            # consts on the (otherwise idle at the head) scalar queue so
            # channel 0's loads are the first sync-queue triggers
            nc.scalar.dma_start(bmat[:], bma.ap()[:, :])
            nc.scalar.dma_start(bmbt[:], bmb.ap()[:, :])
            nc.scalar.dma_start(bb2t[:], bb2.ap()[:, :])
            nc.scalar.dma_start(bblt[:], bbl.ap()[:, :])

            # Static ring. Loads only ever touch the data columns, so the
            # 17+8 zero pads around each 512-col block are zeroed ONCE here
            # (on GpSimd: keeps the DVE queue clear).
            xalls = [
                nc.alloc_sbuf_tensor(f"xall{i}", [128, XALL], bf16).ap()
                for i in range(NBIG)
            ]
            for xa in xalls:
                nc.gpsimd.memset(xa[:, 0:PADF], 0.0)
                mid = bass.AP(
                    tensor=xa.tensor,
                    offset=xa.offset + XW - PADB,
                    ap=[[XALL, 128], [XW, 3], [1, PADF + PADB]],
                )
                nc.gpsimd.memset(mid, 0.0)
                nc.gpsimd.memset(xa[:, XALL - PADB:XALL], 0.0)
            # merged scan outputs: u[:, 537*b+8 : 537*b+520] is block b's
            # window sums (window centered at data col c -> u col 537b+8+c)
            us = [
                nc.alloc_sbuf_tensor(f"u{i}", [128, UW], bf16).ap()
                for i in range(NU)
            ]
            obigs = [
                nc.alloc_sbuf_tensor(f"obig{i}", [128, 4, W], bf16).ap()
                for i in range(NOBIG)
            ]

            SPLIT = 2 * XW  # scan split point in xa (blocks 0-1 | 2-3)

            for c in range(CH):
                xa = xalls[c % NBIG]
                u = us[c % NU]
                og = obigs[c % NOBIG]

                # Three loads per channel (shifted-block layout):
                #   rows 0..119   -> block 0 partitions 8..127
                #   rows 504..511 -> block 0 partitions 0..7 (stash)
                #   rows 120..503 -> blocks 1..3 (partition k = row
                #                    128b - 8 + k), one batched transfer
                nc.sync.dma_start(
                    xa[8:128, PADF:PADF + W], xap[c, 0:120, :]
                )
                nc.sync.dma_start(
                    xa[0:8, PADF:PADF + W], xap[c, 504:512, :]
                )
                src = bass.AP(
                    tensor=x,
                    offset=(c * H + 120) * W,
                    ap=[[W, 128], [128 * W, 3], [1, W]],
                )
                dst = bass.AP(
                    tensor=xa.tensor,
                    offset=xa.offset + XW + PADF,
                    ap=[[XALL, 128], [XW, 3], [1, W]],
                )
                nc.sync.dma_start(dst, src)

                # previous channel's batched store, AFTER this channel's
                # loads in sync-queue order (its sem wait can't block them)
                if c > 0:
                    nc.sync.dma_start(
                        zap[c - 1, :, :].rearrange("(t p) w -> p t w", p=128),
                        obigs[(c - 1) % NOBIG][:, :, :],
                    )

                # Two chained scans, one per block PAIR (0-1 and 2-3): the
                # 17 front-pad zeros of the pair's second block drain the
                # fp32 window state, so one scan instruction crosses the
                # block boundary exactly. Each scan: d0/d1 shifted by PADF,
                # out col (off + t) holds the window ending at xa[off+PADF+t]
                # -> block b rhs = u[:, XW*b+R : XW*b+R+W].
                for off in (0, SPLIT):
                    nc.vector.tensor_tensor_scan(
                        out=u[0:128, off:off + SPLIT - PADF],
                        data0=xa[0:128, off + PADF:off + SPLIT],
                        data1=xa[0:128, off:off + SPLIT - PADF],
                        initial=0.0,
                        op0=mybir.AluOpType.add,
                        op1=mybir.AluOpType.subtract,
                    )

                def rhs_main(t, u=u):
                    return u[0:128, XW * t + R:XW * t + R + W]

                for t in range(4):
                    ps = ppool.tile([128, W], f32)
                    nc.tensor.matmul(
                        ps[0:128, :],
                        (bmat if t == 0 else bmbt)[0:128, 0:128],
                        rhs_main(t),
                        start=True, stop=False, skip_group_check=True,
                    )
                    if t < 3:
                        nc.tensor.matmul(
                            ps[64:128, :], bb2t[0:16, 0:64],
                            u[0:16, XW * (t + 1) + R:XW * (t + 1) + R + W],
                            start=False, stop=True, skip_group_check=True,
                        )
                    else:
                        nc.tensor.matmul(
                            ps[64:128, :], bblt[0:8, 0:64],
                            u[0:8, R:R + W],
                            start=False, stop=True, skip_group_check=True,
                        )
                    nc.scalar.copy(og[:, t, :], ps[0:128, :])
                    if c == CH - 1:
                        # last channel: per-tile stores so the kernel tail
                        # ends on a small transfer
                        nc.sync.dma_start(
                            zap[c, 128 * t:128 * t + 128, :], og[:, t, :]
                        )

    nc.compile()
    _CACHE["nc"] = nc
    return nc


def kernel(tensor: np.ndarray) -> np.ndarray:
    tensor = np.asarray(tensor)
    assert tensor.shape == (NCORES, CH, H, W)
    xb = tensor.astype(ml_dtypes.bfloat16)
    bmaf, bmab, bb2, bbl = _banded()
    nc = _build_program()
    in_maps = [
        {"x": xb[i], "bma": bmaf, "bmb": bmab, "bb2": bb2, "bbl": bbl}
        for i in range(NCORES)
    ]
    res = run_bass_kernel_spmd(nc, in_maps, core_ids=list(range(NCORES)))
    return np.stack(
        [res.results[i]["z"].astype(np.float32) for i in range(NCORES)], axis=0
    )


# revision 4
# speedup vs baseline: 1.2263x; 1.0829x over previous
"""Box filter (radius 8, window 17, zero-padded edges) over dims 2,3 of a
[8, 32, 512, 512] f32 tensor, on 8 Trainium2 NeuronCores.

Decomposition (validated vs the jax reference):
  - Per-axis filter with clipped windows = multiplication by a banded ones
    matrix B (B[i,k] = 1 iff |i-k| <= 8), i.e. Z = B @ X @ B.
  - Column (free-dim) filter: DVE `tensor_tensor_scan` computes the sliding
    window sum via state[t] = (x[t] + state[t-1]) - x[t-17] over a buffer
    with 17 front zeros / 8 back zeros per 512-col block. Measured HW cost
    is 55 + 2.09*N ns (dtype-independent), so block pairs are chained into
    one wide scan each (the 17 front-pad zeros of the pair's second block
    drain the fp32 window state exactly across the boundary). The first and
    last channel use 4 per-block scans instead, shortening pipeline ramp
    and tail.
  - Row (partition-dim) filter: PE matmul. Block t holds input rows
    128t-8..128t+119 (top halo pre-shifted in), so one K=128 banded matmul
    covers an output tile except its last 16 rows; one K<=16 fix matmul
    accumulates those from the next block's partitions 0..15. Rows 504..511
    are stashed in block 0's partitions 0..7 and consumed by tile 3's fix
    (tile 0's main lhsT has rows 0..7 zeroed to ignore the stash).
  - Everything is bf16 end-to-end (input converted on host, output converted
    back): HBM traffic is halved to 16.8 MB in + 16.8 MB out per core, scan
    state stays fp32 inside the DVE, PSUM accumulates f32, and the Scalar
    engine downcasts PSUM->SBUF. Measured rel err ~4e-3 vs the 2e-2 gate.

Engine budget per channel (measured): DVE 2 scans ~4.5 us (critical path),
PE 8 matmuls ~4.4 us, Scalar 4 copies + store issue ~3.4 us, DMA ~2.9 us
across 16 queues. Stores issue on the Scalar queue right after the copies
(program order => no semaphore waits, no sync-queue head-of-line blocking);
the batched input load is split per block pair so scan A can start before
blocks 2-3 arrive; pad memsets run on GpSimd.

Sharding: data-parallel over batch (dim 0) -> 8 cores, one batch each.
"""

import os
import sys

import numpy as np

for _p in ("/opt/trn_rl_repo", "/root/.axon_site/_ro/trn_rl_repo"):
    if os.path.isdir(_p) and _p not in sys.path:
        sys.path.append(_p)

import ml_dtypes

import concourse.bass as bass
import concourse.tile as tile
from concourse import bacc, mybir
from concourse.bass_utils import run_bass_kernel_spmd

R = 8
PADF = 2 * R + 1  # front zero pad (window width)
PADB = R          # back zero pad
H = W = 512
CH = 32
NCORES = 8
XW = PADF + W + PADB  # 537
XALL = 4 * XW         # 2148
UW = XALL - PADF      # 2131: scan output width (u col XW*b+8+c = block b col c)
NBIG = 6
NU = 4
NOBIG = 4

_CACHE = {}


def _banded():
    # Block t partition k holds input row 128t - 8 + k. Main band:
    # |m - (k - 8)| <= 8  ->  k - 16 <= m <= k.
    k = np.arange(128)[:, None]
    m = np.arange(128)[None, :]
    bmain = ((m >= k - 2 * R) & (m <= k)).astype(np.float32)
    # Tile 0's main: partitions 0..7 hold the stashed rows 504..511, not
    # rows -8..-1 — zero them out (zero-pad semantics at the top edge).
    bmainf = bmain.copy()
    bmainf[0:R, :] = 0.0
    # Bottom fix for tile t<3: rhs partitions k (k=0..15) = input row
    # 128t + 120 + k; output rows 64+m (m=0..63, psum slice [64:128]):
    # window iff m + 64 >= k + 112  ->  m >= k + 48.
    kb = np.arange(16)[:, None]
    mb = np.arange(64)[None, :]
    bbot = (mb >= kb + 6 * R).astype(np.float32)
    # Tile 3's fix: rhs partitions k (k=0..7) = stashed row 504+k; same band.
    kl = np.arange(8)[:, None]
    bbotl = (mb >= kl + 6 * R).astype(np.float32)
    bf = ml_dtypes.bfloat16
    return (bmainf.astype(bf), bmain.astype(bf),
            bbot.astype(bf), bbotl.astype(bf))


def _build_program():
    if "nc" in _CACHE:
        return _CACHE["nc"]
    nc = bacc.Bacc(debug=False)
    f32 = mybir.dt.float32
    bf16 = mybir.dt.bfloat16
    x = nc.dram_tensor("x", [CH, H, W], bf16, kind="ExternalInput")
    z = nc.dram_tensor("z", [CH, H, W], bf16, kind="ExternalOutput")
    bma = nc.dram_tensor("bma", [128, 128], bf16, kind="ExternalInput")
    bmb = nc.dram_tensor("bmb", [128, 128], bf16, kind="ExternalInput")
    bb2 = nc.dram_tensor("bb2", [16, 64], bf16, kind="ExternalInput")
    bbl = nc.dram_tensor("bbl", [8, 64], bf16, kind="ExternalInput")
    xap, zap = x.ap(), z.ap()

    with tile.TileContext(nc) as tc:
        with (
            tc.tile_pool(name="consts", bufs=1) as cpool,
            tc.tile_pool(name="psum", bufs=8, space="PSUM") as ppool,
        ):
            bmat = cpool.tile([128, 128], bf16)
            bmbt = cpool.tile([128, 128], bf16)
            bb2t = cpool.tile([16, 64], bf16)
            bblt = cpool.tile([8, 64], bf16)
            # consts on the (otherwise idle at the head) scalar queue so
            # channel 0's loads are the first sync-queue triggers
            nc.scalar.dma_start(bmat[:], bma.ap()[:, :])
            nc.scalar.dma_start(bmbt[:], bmb.ap()[:, :])
            nc.scalar.dma_start(bb2t[:], bb2.ap()[:, :])
            nc.scalar.dma_start(bblt[:], bbl.ap()[:, :])

            # Static ring. Loads only ever touch the data columns, so the
            # 17+8 zero pads around each 512-col block are zeroed ONCE here
            # (on GpSimd: keeps the DVE queue clear).
            xalls = [
                nc.alloc_sbuf_tensor(f"xall{i}", [128, XALL], bf16).ap()
                for i in range(NBIG)
            ]
            for xa in xalls:
                nc.gpsimd.memset(xa[:, 0:PADF], 0.0)
                mid = bass.AP(
                    tensor=xa.tensor,
                    offset=xa.offset + XW - PADB,
                    ap=[[XALL, 128], [XW, 3], [1, PADF + PADB]],
                )
                nc.gpsimd.memset(mid, 0.0)
                nc.gpsimd.memset(xa[:, XALL - PADB:XALL], 0.0)
            us = [
                nc.alloc_sbuf_tensor(f"u{i}", [128, UW], bf16).ap()
                for i in range(NU)
            ]
            obigs = [
                nc.alloc_sbuf_tensor(f"obig{i}", [128, 4, W], bf16).ap()
                for i in range(NOBIG)
            ]

            PAIRW = 2 * XW - PADF  # 1057: width of one block-pair scan

            def scan_part(u, xa, off, width):
                # out col (off+t) = window ending at xa[off+PADF+t]; valid
                # whenever xa[off..off+PADF-1] lie in a zero pad region.
                nc.vector.tensor_tensor_scan(
                    out=u[0:128, off:off + width],
                    data0=xa[0:128, off + PADF:off + PADF + width],
                    data1=xa[0:128, off:off + width],
                    initial=0.0,
                    op0=mybir.AluOpType.add,
                    op1=mybir.AluOpType.subtract,
                )

            for c in range(CH):
                xa = xalls[c % NBIG]
                u = us[c % NU]
                og = obigs[c % NOBIG]

                # Four loads per channel (shifted-block layout):
                #   rows 0..119   -> block 0 partitions 8..127
                #   rows 504..511 -> block 0 partitions 0..7 (stash)
                #   rows 120..247 -> block 1 (gates scan A with block 0)
                #   rows 248..503 -> blocks 2..3 (gates scan B)
                nc.sync.dma_start(
                    xa[8:128, PADF:PADF + W], xap[c, 0:120, :]
                )
                nc.sync.dma_start(
                    xa[0:8, PADF:PADF + W], xap[c, 504:512, :]
                )
                nc.sync.dma_start(
                    xa[:, XW + PADF:XW + PADF + W], xap[c, 120:248, :]
                )
                src = bass.AP(
                    tensor=x,
                    offset=(c * H + 248) * W,
                    ap=[[W, 128], [128 * W, 2], [1, W]],
                )
                dst = bass.AP(
                    tensor=xa.tensor,
                    offset=xa.offset + 2 * XW + PADF,
                    ap=[[XALL, 128], [XW, 2], [1, W]],
                )
                nc.sync.dma_start(dst, src)

                # Column scans. Middle channels: one wide scan per block
                # pair; first/last channel: per-block scans (finer deps ->
                # shorter pipeline ramp and tail).
                if c in (0, CH - 1):
                    for b in range(4):
                        scan_part(u, xa, b * XW, XW - PADF)
                else:
                    scan_part(u, xa, 0, PAIRW)
                    scan_part(u, xa, 2 * XW, PAIRW)

                for t in range(4):
                    ps = ppool.tile([128, W], f32)
                    nc.tensor.matmul(
                        ps[0:128, :],
                        (bmat if t == 0 else bmbt)[0:128, 0:128],
                        u[0:128, XW * t + R:XW * t + R + W],
                        start=True, stop=False, skip_group_check=True,
                    )
                    if t < 3:
                        nc.tensor.matmul(
                            ps[64:128, :], bb2t[0:16, 0:64],
                            u[0:16, XW * (t + 1) + R:XW * (t + 1) + R + W],
                            start=False, stop=True, skip_group_check=True,
                        )
                    else:
                        nc.tensor.matmul(
                            ps[64:128, :], bblt[0:8, 0:64],
                            u[0:8, R:R + W],
                            start=False, stop=True, skip_group_check=True,
                        )
                    nc.scalar.copy(og[:, t, :], ps[0:128, :])
                    if c == CH - 1:
                        # last channel: per-tile stores so the kernel tail
                        # ends on a small transfer (scalar queue: program
                        # order after the copy, no waits)
                        nc.scalar.dma_start(
                            zap[c, 128 * t:128 * t + 128, :], og[:, t, :]
                        )

                if c < CH - 1:
                    # ONE batched 1 MB store; follows the copies on the
                    # scalar queue in program order (no extra waits)
                    nc.scalar.dma_start(
                        zap[c, :, :].rearrange("(t p) w -> p t w", p=128),
                        og[:, :, :],
                    )

    nc.compile()
    _CACHE["nc"] = nc
    return nc


def kernel(tensor: np.ndarray) -> np.ndarray:
    tensor = np.asarray(tensor)
    assert tensor.shape == (NCORES, CH, H, W)
    xb = tensor.astype(ml_dtypes.bfloat16)
    bmaf, bmab, bb2, bbl = _banded()
    nc = _build_program()
    in_maps = [
        {"x": xb[i], "bma": bmaf, "bmb": bmab, "bb2": bb2, "bbl": bbl}
        for i in range(NCORES)
    ]
    res = run_bass_kernel_spmd(nc, in_maps, core_ids=list(range(NCORES)))
    return np.stack(
        [res.results[i]["z"].astype(np.float32) for i in range(NCORES)], axis=0
    )


# revision 5
# speedup vs baseline: 1.2921x; 1.0536x over previous
"""Box filter (radius 8, window 17, zero-padded edges) over dims 2,3 of a
[8, 32, 512, 512] f32 tensor, on 8 Trainium2 NeuronCores.

Decomposition (validated vs the jax reference):
  - Per-axis filter with clipped windows = multiplication by a banded ones
    matrix B (B[i,k] = 1 iff |i-k| <= 8), i.e. Z = B @ X @ B.
  - Column (free-dim) filter: DVE `tensor_tensor_scan` computes the sliding
    window sum via state[t] = (x[t] + state[t-1]) - x[t-17] over a buffer
    with 17 front zeros / 8 back zeros per 512-col block. Measured HW cost
    is 55 + 2.09*N ns (dtype-independent), so block pairs are chained into
    one wide scan each (the 17 front-pad zeros of the pair's second block
    drain the fp32 window state exactly across the boundary). The first and
    last channel use 4 per-block scans instead, shortening pipeline ramp
    and tail.
  - Row (partition-dim) filter: PE matmul. Block t holds input rows
    128t-8..128t+119 (top halo pre-shifted in), so one K=128 banded matmul
    covers an output tile except its last 16 rows; one K<=16 fix matmul
    accumulates those from the next block's partitions 0..15. Rows 504..511
    are stashed in block 0's partitions 0..7 and consumed by tile 3's fix
    (tile 0's main lhsT has rows 0..7 zeroed to ignore the stash).
  - Everything is bf16 end-to-end (input converted on host, output converted
    back): HBM traffic is halved to 16.8 MB in + 16.8 MB out per core, scan
    state stays fp32 inside the DVE, PSUM accumulates f32, and the Scalar
    engine downcasts PSUM->SBUF. Measured rel err ~4e-3 vs the 2e-2 gate.

Engine budget per channel (measured): DVE 2 scans ~4.5 us (critical path),
PE 8 matmuls ~4.4 us, Scalar 4 copies + store issue ~3.4 us, DMA ~2.9 us
across 16 queues. Stores issue on the Scalar queue right after the copies
(program order => no semaphore waits, no sync-queue head-of-line blocking);
the batched input load is split per block pair so scan A can start before
blocks 2-3 arrive; pad memsets run on GpSimd.

Sharding: data-parallel over batch (dim 0) -> 8 cores, one batch each.
"""

import os
import sys

import numpy as np

for _p in ("/opt/trn_rl_repo", "/root/.axon_site/_ro/trn_rl_repo"):
    if os.path.isdir(_p) and _p not in sys.path:
        sys.path.append(_p)

import ml_dtypes

import concourse.bass as bass
import concourse.tile as tile
from concourse import bacc, mybir
from concourse.bass_utils import run_bass_kernel_spmd

R = 8
PADF = 2 * R + 1  # front zero pad (window width)
PADB = R          # back zero pad
H = W = 512
CH = 32
NCORES = 8
XW = PADF + W + PADB  # 537
XALL = 4 * XW         # 2148
UW = XALL - PADF      # 2131: scan output width (u col XW*b+8+c = block b col c)
NBIG = 5
NU = 4
NOBIG = 4

_CACHE = {}


def _banded():
    # Block t partition k holds input row 128t - 8 + k. Main band:
    # |m - (k - 8)| <= 8  ->  k - 16 <= m <= k.
    k = np.arange(128)[:, None]
    m = np.arange(128)[None, :]
    bmain = ((m >= k - 2 * R) & (m <= k)).astype(np.float32)
    # Tile 0's main: partitions 0..7 hold the stashed rows 504..511, not
    # rows -8..-1 — zero them out (zero-pad semantics at the top edge).
    bmainf = bmain.copy()
    bmainf[0:R, :] = 0.0
    # Bottom fix for tile t<3: rhs partitions k (k=0..15) = input row
    # 128t + 120 + k; output rows 64+m (m=0..63, psum slice [64:128]):
    # window iff m + 64 >= k + 112  ->  m >= k + 48.
    kb = np.arange(16)[:, None]
    mb = np.arange(64)[None, :]
    bbot = (mb >= kb + 6 * R).astype(np.float32)
    # Tile 3's fix: rhs partitions k (k=0..7) = stashed row 504+k; same band.
    kl = np.arange(8)[:, None]
    bbotl = (mb >= kl + 6 * R).astype(np.float32)
    bf = ml_dtypes.bfloat16
    return (bmainf.astype(bf), bmain.astype(bf),
            bbot.astype(bf), bbotl.astype(bf))


def _build_program():
    if "nc" in _CACHE:
        return _CACHE["nc"]
    nc = bacc.Bacc(debug=False)
    f32 = mybir.dt.float32
    bf16 = mybir.dt.bfloat16
    x = nc.dram_tensor("x", [CH, H, W], bf16, kind="ExternalInput")
    z = nc.dram_tensor("z", [CH, H, W], bf16, kind="ExternalOutput")
    bma = nc.dram_tensor("bma", [128, 128], bf16, kind="ExternalInput")
    bmb = nc.dram_tensor("bmb", [128, 128], bf16, kind="ExternalInput")
    bb2 = nc.dram_tensor("bb2", [16, 64], bf16, kind="ExternalInput")
    bbl = nc.dram_tensor("bbl", [8, 64], bf16, kind="ExternalInput")
    xap, zap = x.ap(), z.ap()

    with tile.TileContext(nc) as tc:
        with (
            tc.tile_pool(name="consts", bufs=1) as cpool,
            tc.tile_pool(name="psum", bufs=8, space="PSUM") as ppool,
        ):
            bmat = cpool.tile([128, 128], bf16)
            bmbt = cpool.tile([128, 128], bf16)
            bb2t = cpool.tile([16, 64], bf16)
            bblt = cpool.tile([8, 64], bf16)
            # consts on the (otherwise idle at the head) scalar queue so
            # channel 0's loads are the first sync-queue triggers
            nc.scalar.dma_start(bmat[:], bma.ap()[:, :])
            nc.scalar.dma_start(bmbt[:], bmb.ap()[:, :])
            nc.scalar.dma_start(bb2t[:], bb2.ap()[:, :])
            nc.scalar.dma_start(bblt[:], bbl.ap()[:, :])

            # Static ring. Loads only ever touch the data columns, so the
            # 17+8 zero pads around each 512-col block are zeroed ONCE here
            # (on GpSimd: keeps the DVE queue clear).
            xalls = [
                nc.alloc_sbuf_tensor(f"xall{i}", [128, XALL], bf16).ap()
                for i in range(NBIG)
            ]
            for xa in xalls:
                nc.gpsimd.memset(xa[:, 0:PADF], 0.0)
                mid = bass.AP(
                    tensor=xa.tensor,
                    offset=xa.offset + XW - PADB,
                    ap=[[XALL, 128], [XW, 3], [1, PADF + PADB]],
                )
                nc.gpsimd.memset(mid, 0.0)
                nc.gpsimd.memset(xa[:, XALL - PADB:XALL], 0.0)
            us = [
                nc.alloc_sbuf_tensor(f"u{i}", [128, UW], bf16).ap()
                for i in range(NU)
            ]
            obigs = [
                nc.alloc_sbuf_tensor(f"obig{i}", [128, 4, W], bf16).ap()
                for i in range(NOBIG)
            ]

            PAIRW = 2 * XW - PADF  # 1057: width of one block-pair scan

            def scan_part(u, xa, off, width):
                # out col (off+t) = window ending at xa[off+PADF+t]; valid
                # whenever xa[off..off+PADF-1] lie in a zero pad region.
                nc.vector.tensor_tensor_scan(
                    out=u[0:128, off:off + width],
                    data0=xa[0:128, off + PADF:off + PADF + width],
                    data1=xa[0:128, off:off + width],
                    initial=0.0,
                    op0=mybir.AluOpType.add,
                    op1=mybir.AluOpType.subtract,
                )

            for c in range(CH):
                xa = xalls[c % NBIG]
                u = us[c % NU]
                og = obigs[c % NOBIG]

                # Four loads per channel (shifted-block layout):
                #   rows 0..119   -> block 0 partitions 8..127
                #   rows 504..511 -> block 0 partitions 0..7 (stash)
                #   rows 120..247 -> block 1 (gates scan A with block 0)
                #   rows 248..503 -> blocks 2..3 (gates scan B)
                nc.sync.dma_start(
                    xa[8:128, PADF:PADF + W], xap[c, 0:120, :]
                )
                nc.sync.dma_start(
                    xa[0:8, PADF:PADF + W], xap[c, 504:512, :]
                )
                nc.sync.dma_start(
                    xa[:, XW + PADF:XW + PADF + W], xap[c, 120:248, :]
                )
                src = bass.AP(
                    tensor=x,
                    offset=(c * H + 248) * W,
                    ap=[[W, 128], [128 * W, 2], [1, W]],
                )
                dst = bass.AP(
                    tensor=xa.tensor,
                    offset=xa.offset + 2 * XW + PADF,
                    ap=[[XALL, 128], [XW, 2], [1, W]],
                )
                nc.sync.dma_start(dst, src)

                # Column scans. Middle channels: one wide scan per block
                # pair; first/last channel: per-block scans (finer deps ->
                # shorter pipeline ramp and tail).
                if c in (0, CH - 1):
                    for b in range(4):
                        scan_part(u, xa, b * XW, XW - PADF)
                else:
                    scan_part(u, xa, 0, PAIRW)
                    scan_part(u, xa, 2 * XW, PAIRW)

                for t in range(4):
                    ps = ppool.tile([128, W], f32)
                    nc.tensor.matmul(
                        ps[0:128, :],
                        (bmat if t == 0 else bmbt)[0:128, 0:128],
                        u[0:128, XW * t + R:XW * t + R + W],
                        start=True, stop=False, skip_group_check=True,
                    )
                    if t < 3:
                        nc.tensor.matmul(
                            ps[64:128, :], bb2t[0:16, 0:64],
                            u[0:16, XW * (t + 1) + R:XW * (t + 1) + R + W],
                            start=False, stop=True, skip_group_check=True,
                        )
                    else:
                        nc.tensor.matmul(
                            ps[64:128, :], bblt[0:8, 0:64],
                            u[0:8, R:R + W],
                            start=False, stop=True, skip_group_check=True,
                        )
                    nc.scalar.copy(og[:, t, :], ps[0:128, :])
                    if c == CH - 1:
                        # last channel: per-tile stores so the kernel tail
                        # ends on a small transfer (GpSimd/SWDGE queue:
                        # separate descriptor path from the load rings)
                        nc.gpsimd.dma_start(
                            zap[c, 128 * t:128 * t + 128, :], og[:, t, :]
                        )

                if c < CH - 1:
                    # ONE batched store per channel on the otherwise-idle
                    # GpSimd queue: SWDGE descriptors use their own ring, so
                    # a store can never block behind prefetched loads (which
                    # stalled the Scalar queue 15 us when stores shared its
                    # path), and the Scalar sequencer keeps only the copies.
                    nc.gpsimd.dma_start(
                        zap[c, :, :].rearrange("(t p) w -> p t w", p=128),
                        og[:, :, :],
                    )

    nc.compile()
    _CACHE["nc"] = nc
    return nc


def kernel(tensor: np.ndarray) -> np.ndarray:
    tensor = np.asarray(tensor)
    assert tensor.shape == (NCORES, CH, H, W)
    xb = tensor.astype(ml_dtypes.bfloat16)
    bmaf, bmab, bb2, bbl = _banded()
    nc = _build_program()
    in_maps = [
        {"x": xb[i], "bma": bmaf, "bmb": bmab, "bb2": bb2, "bbl": bbl}
        for i in range(NCORES)
    ]
    res = run_bass_kernel_spmd(nc, in_maps, core_ids=list(range(NCORES)))
    return np.stack(
        [res.results[i]["z"].astype(np.float32) for i in range(NCORES)], axis=0
    )
